# revision 4
# baseline (speedup 1.0000x reference)
"""CBAM3D Trainium2 kernel (8 NeuronCores, SPMD).

Reference computation (per batch sample b):
  avg_pool[c] = mean_{d,h,w} x ; max_pool[c] = max_{d,h,w} x
  ca = sigmoid(relu(avg@w1+b1)@w2+b2) + sigmoid(relu(max@w1+b1)@w2+b2)
  refined = x * ca[c]
  P = [mean_c refined, max_c refined]            # [D,H,W,2]
  sa = sigmoid(conv3d_same(P, conv_w))           # 7x7x7x2 -> 1
  out = refined * sa

Sharding: core i handles sample b=i//2, D-half half=i%2 (32 planes, NO host
halo padding). Cross-core traffic: a pair-wise AllGather of channel sum/max
stats (512B) and a pair-wise AllGather of the 3-slot pooled-map halo
(~108KB) — the full-resolution x halo is never re-read from HBM.

KEY THROUGHPUT FACT (cost model + measured): InstTensorTensor only ever
gets the DVE 2x perf mode, but InstTensorScalarPtr (scalar_tensor_tensor
= (in0 op0 scalar) op1 in1, and tensor_scalar) supports 4x_2p — all
operands bf16 + innermost packed (stride 1, count>=2) + SBUF. So every
bulk elementwise op below is an stt with scalar=1.0, running ~2x faster
than the equivalent tensor_tensor. GpSimd bulk ops measured 9-30 G
elem/s (Q7 software) — useless for offload; tensor_reduce ~120 G reads/s
— slower than stt trees.

Per-core pipeline:
  pass1: stream x f32 (plane-pair tiles), cast to a bf16 SBUF cache
         (ACT wh0 / DVE wh1), channel sum via PE matmul vs ones,
         channel max as running stt-max (DVE 4x)
  AllGather stats over {2i,2i+1}; transpose-free tiny MLP on device -> ca
  phase2a per pair: refined = cache*ca in-place (stt 4x); SUM tree C
         64->1 (stt-add 4x) and MAX tree C 64->1 (stt-max 4x) into a
         shared tout[P,2,W]; ONE dual-diagonal routing matmul per slot
         (host-built: sums land on pooled partitions 0:64 = avg rows,
         maxes on 64:128) + 2 ACT psum->pooled copies. Edge pairs first;
         then the pooled halo exchange (AllGather + parity-conditional
         DMAs into the halo slots).
  conv:  49 taps x 5 blocks of accumulating matmuls with host-prebuilt
         band matrices (kh,ci folded into K=128) -> sigmoid -> sa stored
         C-pair-duplicated (innermost len-2 real stride keeps the apply
         stt packed; a stride-0 inner broadcast breaks the perf mode)
  apply: cache *= sa in-place (stt 4x), one DMA per plane-pair to HBM
         bf16
"""

from dataclasses import dataclass

import numpy as np
import ml_dtypes

import concourse.bass as bass
import concourse.tile as tile
import concourse.mybir as mybir
from concourse import bacc, bass_isa

F32 = mybir.dt.float32
BF16 = mybir.dt.bfloat16
AX = mybir.AxisListType
OP = mybir.AluOpType
ACT = mybir.ActivationFunctionType


@dataclass(frozen=True)
class Cfg:
    H: int = 64
    W: int = 64
    C: int = 64
    D_LOC: int = 32          # own planes per core
    HID: int = 4             # C // reduction_ratio
    KS: int = 7
    N_CORES: int = 8
    use_collectives: bool = True
    stop_after: str = "full"   # pass1 | mlp | full

    @property
    def HALO(self):
        return self.KS // 2

    @property
    def S(self):
        return self.D_LOC + 2 * self.HALO   # slots in the pooled map

    @property
    def P(self):
        return 2 * self.H                    # partition dim of pair tiles

    @property
    def WP(self):
        return self.W + 2 * self.HALO        # padded pooled-map width

    @property
    def D_TOT(self):
        return 2 * self.D_LOC                # full-sample depth (2 shards)


FULL = Cfg()


def _bc(ap, shape, axis):
    """broadcast ap (by unsqueezing `axis`) to `shape`"""
    return ap.unsqueeze(axis).broadcast_to(shape)


def build_cbam(nc, cfg: Cfg):
    H, W, C = cfg.H, cfg.W, cfg.C
    P, S, WP, HALO = cfg.P, cfg.S, cfg.WP, cfg.HALO
    D_LOC, HID, KS = cfg.D_LOC, cfg.HID, cfg.KS
    PAIRS = D_LOC // 2
    W2 = W // 2
    NT = KS * KS

    def stt(out, in0, in1, op1):
        """tensor-tensor via InstTensorScalarPtr: (in0*1.0) op1 in1.
        Gets the DVE 4x_2p perf mode (TensorTensor caps at 2x)."""
        nc.vector.scalar_tensor_tensor(out=out, in0=in0, scalar=1.0,
                                       in1=in1, op0=OP.mult, op1=op1)

    xs = nc.dram_tensor("xs", [D_LOC, H, W, C], F32, kind="ExternalInput").ap()
    w1 = nc.dram_tensor("w1", [C, HID], F32, kind="ExternalInput").ap()
    b1t = nc.dram_tensor("b1t", [HID, 1], F32, kind="ExternalInput").ap()
    w2 = nc.dram_tensor("w2", [HID, C], F32, kind="ExternalInput").ap()
    b2 = nc.dram_tensor("b2", [1, C], F32, kind="ExternalInput").ap()
    sband = nc.dram_tensor("sband", [P, NT, H], BF16, kind="ExternalInput").ap()
    routem = nc.dram_tensor("routem", [P, 4, P], BF16, kind="ExternalInput").ap()
    out_t = nc.dram_tensor("out", [D_LOC, H, W, C], BF16, kind="ExternalOutput").ap()

    groups = [[i, i + 1] for i in range(0, cfg.N_CORES, 2)]

    with tile.TileContext(nc) as tc:
        with (
            tc.tile_pool(name="consts", bufs=1) as consts,
            tc.tile_pool(name="cache", bufs=1) as cachep,
            tc.tile_pool(name="stage", bufs=5) as stagep,
            tc.tile_pool(name="tree", bufs=1) as treep,
            tc.tile_pool(name="route", bufs=3) as routep,
            tc.tile_pool(name="work", bufs=2) as workp,
            tc.tile_pool(name="dram", bufs=1, space="DRAM") as dram,
            tc.tile_pool(name="ps_stats", bufs=1, space="PSUM") as ps_stats,
            tc.tile_pool(name="ps_perm", bufs=2, space="PSUM") as ps_perm,
            tc.tile_pool(name="ps_psp", bufs=2, space="PSUM") as ps_psp,
            tc.tile_pool(name="ps_cv", bufs=2, space="PSUM") as ps_cv,
            tc.tile_pool(name="ps_sm", bufs=1, space="PSUM") as ps_sm,
        ):
            # ---------------- constants ----------------
            ones = consts.tile([P, 1], BF16, tag="ones")
            nc.vector.memset(ones, 1.0)

            # routing matrices (host-built, see make_routem)
            rt_sb = consts.tile([P, 4, P], BF16, tag="routem")
            nc.gpsimd.dma_start(
                out=rt_sb[:].rearrange("p i q -> p (i q)"),
                in_=routem.rearrange("p i q -> p (i q)"))

            sband_sb = consts.tile([P, NT, H], BF16, tag="sband")
            nc.gpsimd.dma_start(
                out=sband_sb[:].rearrange("p t h -> p (t h)"),
                in_=sband.rearrange("p t h -> p (t h)"))
            w1_sb = consts.tile([C, HID], F32, tag="w1")
            nc.gpsimd.dma_start(out=w1_sb, in_=w1)
            w2_sb = consts.tile([HID, C], F32, tag="w2")
            nc.gpsimd.dma_start(out=w2_sb, in_=w2)
            b1t_sb = consts.tile([HID, 1], F32, tag="b1t")
            nc.gpsimd.dma_start(out=b1t_sb, in_=b1t)

            def dma_bcast(dst, src_ap, parts):
                a = bass.AP(tensor=src_ap.tensor, offset=src_ap.offset,
                            ap=[[0, parts]] + [list(p) for p in src_ap.ap[1:]])
                nc.gpsimd.dma_start(out=dst, in_=a)

            b2b = consts.tile([2, C], F32, tag="b2")
            dma_bcast(b2b, b2, 2)

            # pre-warm the ACT table set (Relu/Sigmoid) so the first real
            # activation in the latency-critical MLP doesn't pay the load
            warm = consts.tile([1, 1], F32, tag="warm")
            nc.scalar.activation(out=warm, in_=b2b[0:1, 0:1], func=ACT.Relu)
            nc.scalar.activation(out=warm, in_=warm, func=ACT.Sigmoid)
            ones12 = consts.tile([1, 2], F32, tag="ones12")
            nc.vector.memset(ones12, 1.0)

            if cfg.use_collectives:
                wu_s = dram.tile([1, 1], F32, tag="wu_s")
                wu_r = dram.tile([2, 1], F32, tag="wu_r")
                nc.gpsimd.dma_start(out=wu_s, in_=b2b[0:1, 0:1])
                nc.gpsimd.collective_compute(
                    "AllGather", OP.bypass, replica_groups=groups,
                    ins=[wu_s.opt()], outs=[wu_r.opt()])

            # persistent state. pair j covers planes (2j, 2j+1) -> pooled
            # slots (HALO+2j, HALO+2j+1). Halo slots 0:3 / 35:38 come from
            # the neighbor core (or stay zero at sample boundaries).
            cache = [cachep.tile([P, W, C], BF16, tag=f"cache{j}",
                                 name=f"cache{j}") for j in range(PAIRS)]
            W4 = W // 4
            acc_max = cachep.tile([P, W4, C], BF16, tag="acc_max")
            nc.vector.memset(acc_max, -3.0e38)
            pooled = cachep.tile([P, S, WP], BF16, tag="pooled")
            nc.gpsimd.memset(pooled, 0.0)
            # conv blocks (start plane, size): the final 8 planes are two
            # 4-plane blocks so the last tree->conv->apply tail is shorter
            conv_blocks = [(0, 8), (8, 8), (16, 8), (24, 4), (28, 4)]
            sa_sb = [cachep.tile([H, sz, W], BF16, tag=f"sa{b}", name=f"sa{b}")
                     for b, (_, sz) in enumerate(conv_blocks)]
            # sa duplicated along a trailing len-2 axis: the apply stt
            # then reads packed bf16 pairs (keeps the 4x perf mode)
            sa_dup = [cachep.tile([P, sz // 2, W, 2], BF16, tag=f"sad{b}",
                                  name=f"sad{b}")
                      for b, (_, sz) in enumerate(conv_blocks)]

            # ---------------- pass 1: stream + cast + stats ----------------
            # (HWDGE f32 loads + ACT/DVE casts; stt-max keeps DVE at 4x)
            psum_stats = ps_stats.tile([1, 8, C], F32, tag="stats")
            n_wg = W // 8
            mm_i = 0
            n_mm = PAIRS * n_wg
            for j in range(PAIRS):
                for wh in range(2):
                    st = stagep.tile([P, W2, C], F32, tag="stage")
                    nc.sync.dma_start(
                        out=st.rearrange("p w c -> p (w c)"),
                        in_=xs[2 * j:2 * j + 2, :, wh * W2:(wh + 1) * W2, :]
                        .rearrange("d h w c -> (d h) (w c)"))
                    if wh == 0:
                        nc.scalar.copy(
                            out=cache[j][:, 0:W2, :], in_=st)
                    else:
                        nc.vector.tensor_copy(
                            out=cache[j][:, W2:, :], in_=st)
                    # channel max: running stt-max, quarter tiles
                    for qq in range(2):
                        q0 = wh * W2 + qq * W4
                        stt(acc_max[:].rearrange("p w c -> p (w c)"),
                            acc_max[:].rearrange("p w c -> p (w c)"),
                            cache[j][:, q0:q0 + W4, :]
                            .rearrange("p w c -> p (w c)"), OP.max)
                for g in range(n_wg):
                    nc.tensor.matmul(
                        out=psum_stats,
                        lhsT=ones[:, :],
                        rhs=cache[j][:, g * 8:(g + 1) * 8, :],
                        start=(mm_i == 0), stop=(mm_i == n_mm - 1))
                    mm_i += 1

            # finalize stats (mean scale applied here, off the critical path)
            s8 = workp.tile([1, 8, C], F32, tag="s8", bufs=1)
            nc.scalar.copy(out=s8, in_=psum_stats)
            nc.vector.tensor_add(out=s8[:, 0:4, :], in0=s8[:, 0:4, :],
                                 in1=s8[:, 4:8, :])
            nc.vector.tensor_add(out=s8[:, 0:2, :], in0=s8[:, 0:2, :],
                                 in1=s8[:, 2:4, :])
            sumc = workp.tile([1, C], F32, tag="sumc", bufs=1)
            nc.vector.tensor_add(out=sumc, in0=s8[:, 0, :], in1=s8[:, 1, :])
            nc.scalar.mul(out=sumc, in_=sumc,
                          mul=1.0 / float(cfg.D_TOT * H * W))
            # fold acc_max [P, W4, C] over W4 by in-place halving
            wfold = W4
            while wfold > 1:
                wfold //= 2
                stt(acc_max[:, 0:wfold, :].rearrange("p w c -> p (w c)"),
                    acc_max[:, 0:wfold, :].rearrange("p w c -> p (w c)"),
                    acc_max[:, wfold:2 * wfold, :]
                    .rearrange("p w c -> p (w c)"), OP.max)
            maxr = workp.tile([P, C], F32, tag="maxr", bufs=1)
            nc.gpsimd.partition_all_reduce(
                out_ap=maxr, in_ap=acc_max[:, 0, :], channels=P,
                reduce_op=bass_isa.ReduceOp.max)

            snd = dram.tile([2, C], F32, tag="snd")
            rcv = dram.tile([2, 2, C], F32, tag="rcv")
            nc.sync.dma_start(out=snd[0:1, :], in_=sumc)
            nc.sync.dma_start(out=snd[1:2, :], in_=maxr[0:1, :])
            if cfg.use_collectives:
                nc.gpsimd.collective_compute(
                    "AllGather", OP.bypass, replica_groups=groups,
                    ins=[snd.opt()], outs=[rcv.opt()])
            else:
                nc.gpsimd.dma_start(out=rcv[0], in_=snd)
                nc.gpsimd.dma_start(out=rcv[1], in_=snd)

            # ---------------- MLP -> ca (transpose-free) ----------------
            if cfg.stop_after == "pass1":
                return nc
            # land stats transposed: quadT[c, k, r] = rcv[r, k, c]
            quadT = workp.tile([C, 2, 2], F32, tag="quadT", bufs=1)
            for r in range(2):
                nc.sync.dma_start(out=quadT[:, :, r],
                                  in_=rcv[r].rearrange("k c -> c k"))
            pooled2 = workp.tile([C, 2], F32, tag="pooled2", bufs=1)
            nc.vector.tensor_add(out=pooled2[:, 0:1], in0=quadT[:, 0, 0:1],
                                 in1=quadT[:, 0, 1:2])
            nc.vector.tensor_tensor(out=pooled2[:, 1:2], in0=quadT[:, 1, 0:1],
                                    in1=quadT[:, 1, 1:2], op=OP.max)

            psum_h = ps_sm.tile([HID, 2], F32, tag="small")
            nc.tensor.matmul(out=psum_h, lhsT=w1_sb, rhs=pooled2,
                             start=True, stop=True)
            h2 = workp.tile([HID, 2], F32, tag="h2", bufs=1)
            nc.scalar.activation(out=h2, in_=psum_h, func=ACT.Relu,
                                 bias=b1t_sb)
            # psum_ca = h2.T @ w2 + 1x2.T @ b2 (bias folded in as a matmul)
            psum_ca = ps_sm.tile([2, C], F32, tag="small")
            nc.tensor.matmul(out=psum_ca, lhsT=h2, rhs=w2_sb,
                             start=True, stop=False)
            nc.tensor.matmul(out=psum_ca, lhsT=ones12, rhs=b2b[0:1, :],
                             start=False, stop=True)
            ca2 = workp.tile([2, C], BF16, tag="ca2", bufs=1)
            nc.scalar.activation(out=ca2, in_=psum_ca, func=ACT.Sigmoid)
            car = workp.tile([2, C], BF16, tag="car", bufs=1)
            nc.gpsimd.partition_all_reduce(
                out_ap=car, in_ap=ca2, channels=2,
                reduce_op=bass_isa.ReduceOp.add)
            ca_bf = consts.tile([P, C], BF16, tag="ca_bf")
            nc.gpsimd.partition_broadcast(out_ap=ca_bf, in_ap=car[0:1, :])

            # ---------------- phase 2: pooled + conv + apply ----------------
            if cfg.stop_after == "mlp":
                return nc

            def emit_pair_phase2a(j):
                """refine in-place; SUM tree C 64->1 and MAX tree C 64->1
                (all stt, 4x) into tout[P,2,W]; one routing matmul per
                slot + 2 ACT psum->pooled copies."""
                s_e, s_o = HALO + 2 * j, HALO + 2 * j + 1
                stt(cache[j], cache[j], _bc(ca_bf[:, :], [P, W, C], 1),
                    OP.mult)
                tout = routep.tile([P, 2, W], BF16, tag="tout",
                                   name=f"tout{j}")
                # SUM tree: halve C 64 -> 1
                t1s = treep.tile([P, W, C // 2], BF16, tag="t1add",
                                 name=f"t1add_{j}")
                with nc.allow_low_precision(reason="bf16 pooled stats"):
                    stt(t1s, cache[j][:, :, 0:C // 2],
                        cache[j][:, :, C // 2:], OP.add)
                    cf = C // 2
                    while cf > 2:
                        cf //= 2
                        stt(t1s[:, :, 0:cf], t1s[:, :, 0:cf],
                            t1s[:, :, cf:2 * cf], OP.add)
                    stt(tout[:, 0, :], t1s[:, :, 0], t1s[:, :, 1], OP.add)
                # MAX tree: halve C 64 -> 1
                t1m = treep.tile([P, W, C // 2], BF16, tag="t1max",
                                 name=f"t1max_{j}")
                stt(t1m, cache[j][:, :, 0:C // 2],
                    cache[j][:, :, C // 2:], OP.max)
                cf = C // 2
                while cf > 2:
                    cf //= 2
                    stt(t1m[:, :, 0:cf], t1m[:, :, 0:cf],
                        t1m[:, :, cf:2 * cf], OP.max)
                stt(tout[:, 1, :], t1m[:, :, 0], t1m[:, :, 1], OP.max)
                # one dual-diagonal routing matmul per slot: sums land on
                # partitions 0:64, maxes on 64:128 of the pooled map
                for mi, slot, nm in ((0, s_e, "pe"), (1, s_o, "po")):
                    pp = ps_perm.tile([P, 2 * W], F32, tag="perm",
                                      name=f"{nm}{j}")
                    nc.tensor.matmul(out=pp, lhsT=rt_sb[:, mi, :],
                                     rhs=tout[:].rearrange("p r w -> p (r w)"),
                                     start=True, stop=True)
                    nc.scalar.copy(out=pooled[0:H, slot, HALO:HALO + W],
                                   in_=pp[0:H, 0:W])
                    nc.scalar.copy(out=pooled[H:P, slot, HALO:HALO + W],
                                   in_=pp[H:P, W:2 * W])

            # edge pairs first: they feed the pooled-halo exchange
            pair_order = [0, 1, PAIRS - 2, PAIRS - 1] + list(range(2, PAIRS - 2))
            emitted = 0
            while emitted < 4:
                emit_pair_phase2a(pair_order[emitted])
                emitted += 1

            # ---- pooled-map halo exchange (pair-wise) ----
            snd_h = dram.tile([P, 6 * WP], BF16, tag="snd_h")
            rcv_h = dram.tile([2, P, 6 * WP], BF16, tag="rcv_h")
            nc.sync.dma_start(
                out=snd_h[:, 0:3 * WP],
                in_=pooled[:, HALO:2 * HALO, :].rearrange("p s w -> p (s w)"))
            nc.sync.dma_start(
                out=snd_h[:, 3 * WP:],
                in_=pooled[:, S - 2 * HALO:S - HALO, :]
                .rearrange("p s w -> p (s w)"))
            if cfg.use_collectives:
                nc.gpsimd.collective_compute(
                    "AllGather", OP.bypass, replica_groups=groups,
                    ins=[snd_h.opt()], outs=[rcv_h.opt()])
            else:
                nc.gpsimd.dma_start(out=rcv_h[0], in_=snd_h)
                nc.gpsimd.dma_start(out=rcv_h[1], in_=snd_h)
            par = nc.sync.partition_id() & 1
            # half 0: my top halo slots <- neighbor's first 3 own planes
            nc.sync.dma_start(
                out=pooled[:, S - HALO:S, :].rearrange("p s w -> p (s w)"),
                in_=rcv_h[1, :, 0:3 * WP], cond=1 - par)
            # half 1: my low halo slots <- neighbor's last 3 own planes
            nc.sync.dma_start(
                out=pooled[:, 0:HALO, :].rearrange("p s w -> p (s w)"),
                in_=rcv_h[0, :, 3 * WP:], cond=par)

            def emit_conv_blk(blk, start, sz):
                pcv = ps_cv.tile([H, sz, W], F32, tag="cv", name=f"cv{blk}")
                k = 0
                for kd in range(KS):
                    for kw in range(KS):
                        nc.tensor.matmul(
                            out=pcv,
                            lhsT=sband_sb[:, kd * KS + kw, :],
                            rhs=pooled[:, start + kd: start + kd + sz,
                                       kw:kw + W],
                            start=(k == 0), stop=(k == NT - 1),
                            skip_group_check=True)
                        k += 1
                nc.scalar.activation(out=sa_sb[blk], in_=pcv, func=ACT.Sigmoid)
                sa_ev = sa_sb[blk].rearrange("h (a b) w -> h a b w", b=2)
                psp = ps_psp.tile([P, sz // 2, W], F32, tag="psp",
                                  name=f"psp{blk}")
                nc.tensor.matmul(out=psp, lhsT=rt_sb[0:H, 2, :],
                                 rhs=sa_ev[:, :, 0, :], start=True, stop=False)
                nc.tensor.matmul(out=psp, lhsT=rt_sb[0:H, 3, :],
                                 rhs=sa_ev[:, :, 1, :], start=False, stop=True)
                # duplicate along a trailing len-2 axis while copying out
                nc.scalar.copy(
                    out=sa_dup[blk],
                    in_=_bc(psp, [P, sz // 2, W, 2], 3))

            def emit_applies(blk, start, sz):
                for j in range(start // 2, start // 2 + sz // 2):
                    dp = j - start // 2
                    # 4D pattern -> stt (3D-only) can't express it; TT at 2x
                    cv = cache[j].rearrange("p w (a b) -> p w a b", b=2)
                    nc.vector.tensor_tensor(
                        out=cv, in0=cv,
                        in1=_bc(sa_dup[blk][:, dp], [P, W, C // 2, 2], 2),
                        op=OP.mult)
                    nc.sync.dma_start(
                        out=out_t[2 * j:2 * j + 2]
                        .rearrange("d h w c -> (d h) (w c)"),
                        in_=cache[j].rearrange("p w c -> p (w c)"))

            # applies are deferred one conv block: engines run in program
            # order, so an apply emitted right after its conv would stall
            # DVE on the PE pipeline while tree work is still available
            need_emit = [8, 12, 16, 16, 16]  # pairs done before conv blk
            prev = None
            for blk, (start, sz) in enumerate(conv_blocks):
                while emitted < need_emit[blk]:
                    emit_pair_phase2a(pair_order[emitted])
                    emitted += 1
                emit_conv_blk(blk, start, sz)
                if prev is not None:
                    emit_applies(*prev)
                prev = (blk, start, sz)
            emit_applies(*prev)
    return nc


def make_sband(conv_w, cfg: Cfg):
    """Host-side band-matrix construction: [P, KS*KS, H] bf16.

    sband[ci*H+h', kd*KS+kw, h] = conv_w[kd, h'-h+halo, kw, ci] (avg rows
    pre-scaled by 1/C because the pooled map stores channel sums)."""
    H, C, KS, HALO = cfg.H, cfg.C, cfg.KS, cfg.HALO
    cw = np.asarray(conv_w, np.float32)[..., 0]        # [KS,KS,KS,2]
    sb = np.zeros((cfg.P, KS * KS, H), np.float32)
    h = np.arange(H)
    for kd in range(KS):
        for kw in range(KS):
            for ci in range(2):
                scale = (1.0 / C) if ci == 0 else 1.0
                for kh in range(KS):
                    hp = h + kh - HALO                  # h' = h + kh - halo
                    m = (hp >= 0) & (hp < H)
                    sb[ci * H + hp[m], kd * KS + kw, h[m]] = cw[kd, kh, kw, ci] * scale
    return sb.astype(ml_dtypes.bfloat16)


def make_routem(cfg: Cfg):
    """Routing matrices [P, 4, P] bf16 (lhsT convention: out[q] sums
    lhsT[p, q] * rhs[p]).

    i=0 (m_e):  p<64  -> cols {p, p+64}   even-slot sum+max router
    i=1 (m_o):  p>=64 -> cols {p-64, p}   odd-slot sum+max router
    i=2 (qa_e): p<64  -> col p            sa even planes -> partitions 0:64
    i=3 (qb_e): p<64  -> col p+64         sa odd planes -> partitions 64:128
    """
    P, H = cfg.P, cfg.H
    rm = np.zeros((P, 4, P), np.float32)
    h = np.arange(H)
    rm[h, 0, h] = 1.0
    rm[h, 0, h + H] = 1.0
    rm[H + h, 1, h] = 1.0
    rm[H + h, 1, H + h] = 1.0
    rm[h, 2, h] = 1.0
    rm[h, 3, h + H] = 1.0
    return rm.astype(ml_dtypes.bfloat16)


def make_core_inputs(x, w1, b1, w2, b2, sband_np, routem_np, cfg: Cfg):
    """Shard the full inputs into per-core in_maps (no halo padding)."""
    C, D_LOC = cfg.C, cfg.D_LOC
    x = np.ascontiguousarray(np.asarray(x, np.float32))
    in_maps = []
    for core in range(cfg.N_CORES):
        b, half = core // 2, core % 2
        d0 = half * D_LOC
        in_maps.append({
            "xs": x[b, d0:d0 + D_LOC],
            "w1": np.asarray(w1, np.float32).reshape(C, cfg.HID),
            "b1t": np.asarray(b1, np.float32).reshape(cfg.HID, 1),
            "w2": np.asarray(w2, np.float32).reshape(cfg.HID, C),
            "b2": np.asarray(b2, np.float32).reshape(1, C),
            "sband": sband_np,
            "routem": routem_np,
        })
    return in_maps


_COMPILED = {}


def get_compiled(cfg: Cfg = FULL):
    if cfg not in _COMPILED:
        nc = bacc.Bacc("TRN2", target_bir_lowering=False, debug=False,
                       num_devices=cfg.N_CORES)
        build_cbam(nc, cfg)
        nc.compile()
        _COMPILED[cfg] = nc
    return _COMPILED[cfg]


def kernel(x, w1, b1, w2, b2, conv_w):
    from concourse.bass_utils import run_bass_kernel_spmd

    cfg = FULL
    nc = get_compiled(cfg)
    sband_np = make_sband(conv_w, cfg)
    routem_np = make_routem(cfg)
    in_maps = make_core_inputs(x, w1, b1, w2, b2, sband_np, routem_np, cfg)
    res = run_bass_kernel_spmd(nc, in_maps, list(range(cfg.N_CORES)))
    B, D = 4, 64
    out = np.empty((B, D, cfg.H, cfg.W, cfg.C), np.float32)
    for core in range(cfg.N_CORES):
        b, half = core // 2, core % 2
        d0 = half * cfg.D_LOC
        out[b, d0:d0 + cfg.D_LOC] = np.asarray(
            res.results[core]["out"], dtype=np.float32)
    return out


# revision 8
# speedup vs baseline: 1.3521x; 1.3521x over previous
"""CBAM3D Trainium2 kernel (8 NeuronCores, SPMD).

Reference computation (per batch sample b):
  avg_pool[c] = mean_{d,h,w} x ; max_pool[c] = max_{d,h,w} x
  ca = sigmoid(relu(avg@w1+b1)@w2+b2) + sigmoid(relu(max@w1+b1)@w2+b2)
  refined = x * ca[c]
  P = [mean_c refined, max_c refined]            # [D,H,W,2]
  sa = sigmoid(conv3d_same(P, conv_w))           # 7x7x7x2 -> 1
  out = refined * sa

Sharding: core i handles sample b=i//2, D-half half=i%2 (32 planes, NO host
halo padding). Cross-core traffic: a pair-wise AllGather of channel sum/max
stats (512B) and a pair-wise AllGather of the 3-slot pooled-map halo
(~108KB) — the full-resolution x halo is never re-read from HBM.

MEASURED ENGINE FACTS (HW, not the CoreSim model):
- DVE TensorTensor bf16 packed = 2x mode ~238 G out-elem/s; this is the
  ceiling. scalar_tensor_tensor (TensorScalarPtr) runs at 1x on real HW
  despite the cost model advertising 4x_2p — do NOT use it for bulk work.
- GpSimd bulk copy/tensor_scalar: 9-30 G elem/s (Q7 software) and the
  Pool engine REJECTS TensorTensor at the ISA level — no offload there.
- vector.tensor_reduce: ~120 G reads/s — slower than a TT halving tree.
- In-place running acc TT (out==in0) measured 190 G vs 238 out-of-place;
  ping-pong buffers recover the difference.
- ACT (scalar) engine: ~141 G elem/s copies, per-partition scale only.

Per-core pipeline:
  pass1: stream x f32 (plane-pair tiles), cast to a bf16 SBUF cache
         (ACT wh0 / DVE wh1), channel sum via PE matmul vs ones,
         channel max as a ping-pong TT-max (DVE)
  AllGather stats over {2i,2i+1}; transpose-free tiny MLP on device -> ca
  phase2a per pair: refined = cache*ca in-place (TT); SUM and MAX
         trees C 64->4 (compact out-of-place TT chain) + tensor_reduce
         4->1 into a shared tout[P,2,W]; ONE dual-diagonal routing
         matmul per slot
         (host-built: sums land on pooled partitions 0:64 = avg rows,
         maxes on 64:128) + 2 ACT psum->pooled copies. Edge pairs first;
         then the pooled halo exchange (AllGather + parity-conditional
         DMAs into the halo slots).
  conv:  49 taps x 5 blocks of accumulating matmuls with host-prebuilt
         band matrices (kh,ci folded into K=128) -> sigmoid -> sa stored
         C-pair-duplicated (innermost len-2 real stride keeps the apply
         TT in 2x mode; a stride-0 inner broadcast breaks it)
  apply: cache *= sa in-place (TT 2x), one DMA per plane-pair to HBM
         bf16
"""

from dataclasses import dataclass

import numpy as np
import ml_dtypes

import concourse.bass as bass
import concourse.tile as tile
import concourse.mybir as mybir
from concourse import bacc, bass_isa

F32 = mybir.dt.float32
BF16 = mybir.dt.bfloat16
AX = mybir.AxisListType
OP = mybir.AluOpType
ACT = mybir.ActivationFunctionType


@dataclass(frozen=True)
class Cfg:
    H: int = 64
    W: int = 64
    C: int = 64
    D_LOC: int = 32          # own planes per core
    HID: int = 4             # C // reduction_ratio
    KS: int = 7
    N_CORES: int = 8
    use_collectives: bool = True
    stop_after: str = "full"   # pass1 | mlp | full

    @property
    def HALO(self):
        return self.KS // 2

    @property
    def S(self):
        return self.D_LOC + 2 * self.HALO   # slots in the pooled map

    @property
    def P(self):
        return 2 * self.H                    # partition dim of pair tiles

    @property
    def WP(self):
        return self.W + 2 * self.HALO        # padded pooled-map width

    @property
    def D_TOT(self):
        return 2 * self.D_LOC                # full-sample depth (2 shards)


FULL = Cfg()


def _bc(ap, shape, axis):
    """broadcast ap (by unsqueezing `axis`) to `shape`"""
    return ap.unsqueeze(axis).broadcast_to(shape)


def build_cbam(nc, cfg: Cfg):
    H, W, C = cfg.H, cfg.W, cfg.C
    P, S, WP, HALO = cfg.P, cfg.S, cfg.WP, cfg.HALO
    D_LOC, HID, KS = cfg.D_LOC, cfg.HID, cfg.KS
    PAIRS = D_LOC // 2
    W2 = W // 2
    NT = KS * KS

    def tt(out, in0, in1, op):
        nc.vector.tensor_tensor(out=out, in0=in0, in1=in1, op=op)

    xs = nc.dram_tensor("xs", [D_LOC, H, W, C], F32, kind="ExternalInput").ap()
    w1 = nc.dram_tensor("w1", [C, HID], F32, kind="ExternalInput").ap()
    b1t = nc.dram_tensor("b1t", [HID, 1], F32, kind="ExternalInput").ap()
    w2 = nc.dram_tensor("w2", [HID, C], F32, kind="ExternalInput").ap()
    b2 = nc.dram_tensor("b2", [1, C], F32, kind="ExternalInput").ap()
    sband = nc.dram_tensor("sband", [P, NT, H], BF16, kind="ExternalInput").ap()
    routem = nc.dram_tensor("routem", [P, 4, P], BF16, kind="ExternalInput").ap()
    out_t = nc.dram_tensor("out", [D_LOC, H, W, C], BF16, kind="ExternalOutput").ap()

    groups = [[i, i + 1] for i in range(0, cfg.N_CORES, 2)]

    with tile.TileContext(nc) as tc:
        with (
            tc.tile_pool(name="consts", bufs=1) as consts,
            tc.tile_pool(name="cache", bufs=1) as cachep,
            tc.tile_pool(name="stage", bufs=4) as stagep,
            tc.tile_pool(name="tree", bufs=1) as treep,
            tc.tile_pool(name="route", bufs=3) as routep,
            tc.tile_pool(name="work", bufs=2) as workp,
            tc.tile_pool(name="dram", bufs=1, space="DRAM") as dram,
            tc.tile_pool(name="ps_stats", bufs=1, space="PSUM") as ps_stats,
            tc.tile_pool(name="ps_perm", bufs=2, space="PSUM") as ps_perm,
            tc.tile_pool(name="ps_psp", bufs=2, space="PSUM") as ps_psp,
            tc.tile_pool(name="ps_cv", bufs=2, space="PSUM") as ps_cv,
            tc.tile_pool(name="ps_sm", bufs=1, space="PSUM") as ps_sm,
        ):
            # ---------------- constants ----------------
            ones = consts.tile([P, 1], BF16, tag="ones")
            nc.vector.memset(ones, 1.0)

            # routing matrices (host-built, see make_routem)
            rt_sb = consts.tile([P, 4, P], BF16, tag="routem")
            nc.gpsimd.dma_start(
                out=rt_sb[:].rearrange("p i q -> p (i q)"),
                in_=routem.rearrange("p i q -> p (i q)"))

            sband_sb = consts.tile([P, NT, H], BF16, tag="sband")
            nc.gpsimd.dma_start(
                out=sband_sb[:].rearrange("p t h -> p (t h)"),
                in_=sband.rearrange("p t h -> p (t h)"))
            w1_sb = consts.tile([C, HID], F32, tag="w1")
            nc.gpsimd.dma_start(out=w1_sb, in_=w1)
            w2_sb = consts.tile([HID, C], F32, tag="w2")
            nc.gpsimd.dma_start(out=w2_sb, in_=w2)
            b1t_sb = consts.tile([HID, 1], F32, tag="b1t")
            nc.gpsimd.dma_start(out=b1t_sb, in_=b1t)

            def dma_bcast(dst, src_ap, parts):
                a = bass.AP(tensor=src_ap.tensor, offset=src_ap.offset,
                            ap=[[0, parts]] + [list(p) for p in src_ap.ap[1:]])
                nc.gpsimd.dma_start(out=dst, in_=a)

            b2b = consts.tile([2, C], F32, tag="b2")
            dma_bcast(b2b, b2, 2)

            # pre-warm the ACT table set (Relu/Sigmoid) so the first real
            # activation in the latency-critical MLP doesn't pay the load
            warm = consts.tile([1, 1], F32, tag="warm")
            nc.scalar.activation(out=warm, in_=b2b[0:1, 0:1], func=ACT.Relu)
            nc.scalar.activation(out=warm, in_=warm, func=ACT.Sigmoid)
            ones12 = consts.tile([1, 2], F32, tag="ones12")
            nc.vector.memset(ones12, 1.0)

            if cfg.use_collectives:
                wu_s = dram.tile([1, 1], F32, tag="wu_s")
                wu_r = dram.tile([2, 1], F32, tag="wu_r")
                nc.gpsimd.dma_start(out=wu_s, in_=b2b[0:1, 0:1])
                nc.gpsimd.collective_compute(
                    "AllGather", OP.bypass, replica_groups=groups,
                    ins=[wu_s.opt()], outs=[wu_r.opt()])

            # persistent state. pair j covers planes (2j, 2j+1) -> pooled
            # slots (HALO+2j, HALO+2j+1). Halo slots 0:3 / 35:38 come from
            # the neighbor core (or stay zero at sample boundaries).
            cache = [cachep.tile([P, W, C], BF16, tag=f"cache{j}",
                                 name=f"cache{j}") for j in range(PAIRS)]
            W4 = W // 4
            acc_max = [cachep.tile([P, W4, C], BF16, tag=f"acc_max{i}",
                                   name=f"acc_max{i}")
                       for i in range(2)]
            nc.vector.memset(acc_max[1], -3.0e38)
            pooled = cachep.tile([P, S, WP], BF16, tag="pooled")
            nc.gpsimd.memset(pooled, 0.0)
            # conv blocks (start plane, size): the final 8 planes are two
            # 4-plane blocks so the last tree->conv->apply tail is shorter
            conv_blocks = [(0, 8), (8, 8), (16, 8), (24, 6), (30, 2)]
            sa_sb = [cachep.tile([H, sz, W], BF16, tag=f"sa{b}", name=f"sa{b}")
                     for b, (_, sz) in enumerate(conv_blocks)]
            # sa duplicated along a trailing len-2 axis: the apply stt
            # then reads packed bf16 pairs (keeps the 4x perf mode)
            sa_dup = [cachep.tile([P, sz // 2, W, 2], BF16, tag=f"sad{b}",
                                  name=f"sad{b}")
                      for b, (_, sz) in enumerate(conv_blocks)]

            # ---------------- pass 1: stream + cast + stats ----------------
            # (HWDGE f32 loads + ACT/DVE casts; stt-max keeps DVE at 4x)
            psum_stats = ps_stats.tile([1, 8, C], F32, tag="stats")
            n_wg = W // 8
            acc_i = 0
            mm_i = 0
            n_mm = PAIRS * n_wg
            for j in range(PAIRS):
                for wh in range(2):
                    st = stagep.tile([P, W2, C], F32, tag="stage")
                    nc.sync.dma_start(
                        out=st.rearrange("p w c -> p (w c)"),
                        in_=xs[2 * j:2 * j + 2, :, wh * W2:(wh + 1) * W2, :]
                        .rearrange("d h w c -> (d h) (w c)"))
                    if wh == 0:
                        nc.scalar.copy(
                            out=cache[j][:, 0:W2, :], in_=st)
                    else:
                        nc.vector.tensor_copy(
                            out=cache[j][:, W2:, :], in_=st)
                    # channel max: ping-pong running TT-max, quarter tiles
                    for qq in range(2):
                        q0 = wh * W2 + qq * W4
                        dst = acc_max[acc_i % 2]
                        srp = acc_max[(acc_i + 1) % 2]
                        tt(dst[:].rearrange("p w c -> p (w c)"),
                           srp[:].rearrange("p w c -> p (w c)"),
                           cache[j][:, q0:q0 + W4, :]
                           .rearrange("p w c -> p (w c)"), OP.max)
                        acc_i += 1
                for g in range(n_wg):
                    nc.tensor.matmul(
                        out=psum_stats,
                        lhsT=ones[:, :],
                        rhs=cache[j][:, g * 8:(g + 1) * 8, :],
                        start=(mm_i == 0), stop=(mm_i == n_mm - 1))
                    mm_i += 1

            # finalize stats (mean scale applied here, off the critical path)
            s8 = workp.tile([1, 8, C], F32, tag="s8", bufs=1)
            nc.scalar.copy(out=s8, in_=psum_stats)
            nc.vector.tensor_add(out=s8[:, 0:4, :], in0=s8[:, 0:4, :],
                                 in1=s8[:, 4:8, :])
            nc.vector.tensor_add(out=s8[:, 0:2, :], in0=s8[:, 0:2, :],
                                 in1=s8[:, 2:4, :])
            sumc = workp.tile([1, C], F32, tag="sumc", bufs=1)
            nc.vector.tensor_add(out=sumc, in0=s8[:, 0, :], in1=s8[:, 1, :])
            nc.scalar.mul(out=sumc, in_=sumc,
                          mul=1.0 / float(cfg.D_TOT * H * W))
            # fold acc [P, W4, C] over W4, ping-pong halving
            am = acc_max[(acc_i + 1) % 2]
            ao = acc_max[acc_i % 2]
            wfold = W4
            while wfold > 1:
                wfold //= 2
                tt(ao[:, 0:wfold, :].rearrange("p w c -> p (w c)"),
                   am[:, 0:wfold, :].rearrange("p w c -> p (w c)"),
                   am[:, wfold:2 * wfold, :]
                   .rearrange("p w c -> p (w c)"), OP.max)
                am, ao = ao, am
            maxr = workp.tile([P, C], F32, tag="maxr", bufs=1)
            nc.gpsimd.partition_all_reduce(
                out_ap=maxr, in_ap=am[:, 0, :], channels=P,
                reduce_op=bass_isa.ReduceOp.max)

            snd = dram.tile([2, C], F32, tag="snd")
            rcv = dram.tile([2, 2, C], F32, tag="rcv")
            nc.sync.dma_start(out=snd[0:1, :], in_=sumc)
            nc.sync.dma_start(out=snd[1:2, :], in_=maxr[0:1, :])
            if cfg.use_collectives:
                nc.gpsimd.collective_compute(
                    "AllGather", OP.bypass, replica_groups=groups,
                    ins=[snd.opt()], outs=[rcv.opt()])
            else:
                nc.gpsimd.dma_start(out=rcv[0], in_=snd)
                nc.gpsimd.dma_start(out=rcv[1], in_=snd)

            # ---------------- MLP -> ca (transpose-free) ----------------
            if cfg.stop_after == "pass1":
                return nc
            # land stats transposed: quadT[c, k, r] = rcv[r, k, c]
            quadT = workp.tile([C, 2, 2], F32, tag="quadT", bufs=1)
            for r in range(2):
                nc.sync.dma_start(out=quadT[:, :, r],
                                  in_=rcv[r].rearrange("k c -> c k"))
            pooled2 = workp.tile([C, 2], F32, tag="pooled2", bufs=1)
            nc.vector.tensor_add(out=pooled2[:, 0:1], in0=quadT[:, 0, 0:1],
                                 in1=quadT[:, 0, 1:2])
            nc.vector.tensor_tensor(out=pooled2[:, 1:2], in0=quadT[:, 1, 0:1],
                                    in1=quadT[:, 1, 1:2], op=OP.max)

            psum_h = ps_sm.tile([HID, 2], F32, tag="small")
            nc.tensor.matmul(out=psum_h, lhsT=w1_sb, rhs=pooled2,
                             start=True, stop=True)
            h2 = workp.tile([HID, 2], F32, tag="h2", bufs=1)
            nc.scalar.activation(out=h2, in_=psum_h, func=ACT.Relu,
                                 bias=b1t_sb)
            # psum_ca = h2.T @ w2 + 1x2.T @ b2 (bias folded in as a matmul)
            psum_ca = ps_sm.tile([2, C], F32, tag="small")
            nc.tensor.matmul(out=psum_ca, lhsT=h2, rhs=w2_sb,
                             start=True, stop=False)
            nc.tensor.matmul(out=psum_ca, lhsT=ones12, rhs=b2b[0:1, :],
                             start=False, stop=True)
            ca2 = workp.tile([2, C], BF16, tag="ca2", bufs=1)
            nc.scalar.activation(out=ca2, in_=psum_ca, func=ACT.Sigmoid)
            car = workp.tile([2, C], BF16, tag="car", bufs=1)
            nc.gpsimd.partition_all_reduce(
                out_ap=car, in_ap=ca2, channels=2,
                reduce_op=bass_isa.ReduceOp.add)
            ca_bf = consts.tile([P, C], BF16, tag="ca_bf")
            nc.gpsimd.partition_broadcast(out_ap=ca_bf, in_ap=car[0:1, :])

            # ---------------- phase 2: pooled + conv + apply ----------------
            if cfg.stop_after == "mlp":
                return nc

            def emit_pair_phase2a(j):
                """refine in-place; SUM tree C 64->1 and MAX tree C 64->1
                (all stt, 4x) into tout[P,2,W]; one routing matmul per
                slot + 2 ACT psum->pooled copies."""
                s_e, s_o = HALO + 2 * j, HALO + 2 * j + 1
                tt(cache[j], cache[j], _bc(ca_bf[:, :], [P, W, C], 1),
                   OP.mult)
                tout = routep.tile([P, 2, W], BF16, tag="tout",
                                   name=f"tout{j}")
                # compact out-of-place halving trees C 64 -> 4, then a
                # single tensor_reduce 4 -> 1 into tout
                with nc.allow_low_precision(reason="bf16 pooled stats"):
                    for row, op, rop, tg in ((0, OP.add, OP.add, "s"),
                                             (1, OP.max, OP.max, "m")):
                        prev = cache[j]
                        cf = C
                        while cf > 4:
                            cf //= 2
                            nxt = treep.tile([P, W, cf], BF16,
                                             tag=f"t{tg}{cf}",
                                             name=f"t{tg}{cf}_{j}")
                            tt(nxt, prev[:, :, 0:cf], prev[:, :, cf:2 * cf],
                               op)
                            prev = nxt
                        nc.vector.tensor_reduce(
                            out=tout[:, row, :], in_=prev, axis=AX.X, op=rop)
                # one dual-diagonal routing matmul per slot: sums land on
                # partitions 0:64, maxes on 64:128 of the pooled map
                for mi, slot, nm in ((0, s_e, "pe"), (1, s_o, "po")):
                    pp = ps_perm.tile([P, 2 * W], F32, tag="perm",
                                      name=f"{nm}{j}")
                    nc.tensor.matmul(out=pp, lhsT=rt_sb[:, mi, :],
                                     rhs=tout[:].rearrange("p r w -> p (r w)"),
                                     start=True, stop=True)
                    nc.scalar.copy(out=pooled[0:H, slot, HALO:HALO + W],
                                   in_=pp[0:H, 0:W])
                    nc.scalar.copy(out=pooled[H:P, slot, HALO:HALO + W],
                                   in_=pp[H:P, W:2 * W])

            # edge pairs first: they feed the pooled-halo exchange
            pair_order = [0, 1, PAIRS - 2, PAIRS - 1] + list(range(2, PAIRS - 2))
            emitted = 0
            while emitted < 4:
                emit_pair_phase2a(pair_order[emitted])
                emitted += 1

            # ---- pooled-map halo exchange (pair-wise) ----
            snd_h = dram.tile([P, 6 * WP], BF16, tag="snd_h")
            rcv_h = dram.tile([2, P, 6 * WP], BF16, tag="rcv_h")
            nc.sync.dma_start(
                out=snd_h[:, 0:3 * WP],
                in_=pooled[:, HALO:2 * HALO, :].rearrange("p s w -> p (s w)"))
            nc.sync.dma_start(
                out=snd_h[:, 3 * WP:],
                in_=pooled[:, S - 2 * HALO:S - HALO, :]
                .rearrange("p s w -> p (s w)"))
            if cfg.use_collectives:
                nc.gpsimd.collective_compute(
                    "AllGather", OP.bypass, replica_groups=groups,
                    ins=[snd_h.opt()], outs=[rcv_h.opt()])
            else:
                nc.gpsimd.dma_start(out=rcv_h[0], in_=snd_h)
                nc.gpsimd.dma_start(out=rcv_h[1], in_=snd_h)
            par = nc.sync.partition_id() & 1
            # half 0: my top halo slots <- neighbor's first 3 own planes
            nc.sync.dma_start(
                out=pooled[:, S - HALO:S, :].rearrange("p s w -> p (s w)"),
                in_=rcv_h[1, :, 0:3 * WP], cond=1 - par)
            # half 1: my low halo slots <- neighbor's last 3 own planes
            nc.sync.dma_start(
                out=pooled[:, 0:HALO, :].rearrange("p s w -> p (s w)"),
                in_=rcv_h[0, :, 3 * WP:], cond=par)

            def emit_conv_blk(blk, start, sz):
                pcv = ps_cv.tile([H, sz, W], F32, tag="cv", name=f"cv{blk}")
                k = 0
                for kd in range(KS):
                    for kw in range(KS):
                        nc.tensor.matmul(
                            out=pcv,
                            lhsT=sband_sb[:, kd * KS + kw, :],
                            rhs=pooled[:, start + kd: start + kd + sz,
                                       kw:kw + W],
                            start=(k == 0), stop=(k == NT - 1),
                            skip_group_check=True)
                        k += 1
                nc.scalar.activation(out=sa_sb[blk], in_=pcv, func=ACT.Sigmoid)
                sa_ev = sa_sb[blk].rearrange("h (a b) w -> h a b w", b=2)
                psp = ps_psp.tile([P, sz // 2, W], F32, tag="psp",
                                  name=f"psp{blk}")
                nc.tensor.matmul(out=psp, lhsT=rt_sb[0:H, 2, :],
                                 rhs=sa_ev[:, :, 0, :], start=True, stop=False)
                nc.tensor.matmul(out=psp, lhsT=rt_sb[0:H, 3, :],
                                 rhs=sa_ev[:, :, 1, :], start=False, stop=True)
                # duplicate along a trailing len-2 axis while copying out
                nc.scalar.copy(
                    out=sa_dup[blk],
                    in_=_bc(psp, [P, sz // 2, W, 2], 3))

            def emit_applies(blk, start, sz):
                for j in range(start // 2, start // 2 + sz // 2):
                    dp = j - start // 2
                    # 4D pattern -> stt (3D-only) can't express it; TT at 2x
                    cv = cache[j].rearrange("p w (a b) -> p w a b", b=2)
                    nc.vector.tensor_tensor(
                        out=cv, in0=cv,
                        in1=_bc(sa_dup[blk][:, dp], [P, W, C // 2, 2], 2),
                        op=OP.mult)
                    nc.sync.dma_start(
                        out=out_t[2 * j:2 * j + 2]
                        .rearrange("d h w c -> (d h) (w c)"),
                        in_=cache[j].rearrange("p w c -> p (w c)"))

            # applies are deferred one conv block: engines run in program
            # order, so an apply emitted right after its conv would stall
            # DVE on the PE pipeline while tree work is still available
            need_emit = [8, 12, 16, 16, 16]  # pairs done before conv blk
            prev = None
            for blk, (start, sz) in enumerate(conv_blocks):
                while emitted < need_emit[blk]:
                    emit_pair_phase2a(pair_order[emitted])
                    emitted += 1
                emit_conv_blk(blk, start, sz)
                if prev is not None:
                    emit_applies(*prev)
                prev = (blk, start, sz)
            emit_applies(*prev)
    return nc


def make_sband(conv_w, cfg: Cfg):
    """Host-side band-matrix construction: [P, KS*KS, H] bf16.

    sband[ci*H+h', kd*KS+kw, h] = conv_w[kd, h'-h+halo, kw, ci] (avg rows
    pre-scaled by 1/C because the pooled map stores channel sums)."""
    H, C, KS, HALO = cfg.H, cfg.C, cfg.KS, cfg.HALO
    cw = np.asarray(conv_w, np.float32)[..., 0]        # [KS,KS,KS,2]
    sb = np.zeros((cfg.P, KS * KS, H), np.float32)
    h = np.arange(H)
    for kd in range(KS):
        for kw in range(KS):
            for ci in range(2):
                scale = (1.0 / C) if ci == 0 else 1.0
                for kh in range(KS):
                    hp = h + kh - HALO                  # h' = h + kh - halo
                    m = (hp >= 0) & (hp < H)
                    sb[ci * H + hp[m], kd * KS + kw, h[m]] = cw[kd, kh, kw, ci] * scale
    return sb.astype(ml_dtypes.bfloat16)


def make_routem(cfg: Cfg):
    """Routing matrices [P, 4, P] bf16 (lhsT convention: out[q] sums
    lhsT[p, q] * rhs[p]).

    i=0 (m_e):  p<64  -> cols {p, p+64}   even-slot sum+max router
    i=1 (m_o):  p>=64 -> cols {p-64, p}   odd-slot sum+max router
    i=2 (qa_e): p<64  -> col p            sa even planes -> partitions 0:64
    i=3 (qb_e): p<64  -> col p+64         sa odd planes -> partitions 64:128
    """
    P, H = cfg.P, cfg.H
    rm = np.zeros((P, 4, P), np.float32)
    h = np.arange(H)
    rm[h, 0, h] = 1.0
    rm[h, 0, h + H] = 1.0
    rm[H + h, 1, h] = 1.0
    rm[H + h, 1, H + h] = 1.0
    rm[h, 2, h] = 1.0
    rm[h, 3, h + H] = 1.0
    return rm.astype(ml_dtypes.bfloat16)


def make_core_inputs(x, w1, b1, w2, b2, sband_np, routem_np, cfg: Cfg):
    """Shard the full inputs into per-core in_maps (no halo padding)."""
    C, D_LOC = cfg.C, cfg.D_LOC
    x = np.ascontiguousarray(np.asarray(x, np.float32))
    in_maps = []
    for core in range(cfg.N_CORES):
        b, half = core // 2, core % 2
        d0 = half * D_LOC
        in_maps.append({
            "xs": x[b, d0:d0 + D_LOC],
            "w1": np.asarray(w1, np.float32).reshape(C, cfg.HID),
            "b1t": np.asarray(b1, np.float32).reshape(cfg.HID, 1),
            "w2": np.asarray(w2, np.float32).reshape(cfg.HID, C),
            "b2": np.asarray(b2, np.float32).reshape(1, C),
            "sband": sband_np,
            "routem": routem_np,
        })
    return in_maps


_COMPILED = {}


def get_compiled(cfg: Cfg = FULL):
    if cfg not in _COMPILED:
        nc = bacc.Bacc("TRN2", target_bir_lowering=False, debug=False,
                       num_devices=cfg.N_CORES)
        build_cbam(nc, cfg)
        nc.compile()
        _COMPILED[cfg] = nc
    return _COMPILED[cfg]


def kernel(x, w1, b1, w2, b2, conv_w):
    from concourse.bass_utils import run_bass_kernel_spmd

    cfg = FULL
    nc = get_compiled(cfg)
    sband_np = make_sband(conv_w, cfg)
    routem_np = make_routem(cfg)
    in_maps = make_core_inputs(x, w1, b1, w2, b2, sband_np, routem_np, cfg)
    res = run_bass_kernel_spmd(nc, in_maps, list(range(cfg.N_CORES)))
    B, D = 4, 64
    out = np.empty((B, D, cfg.H, cfg.W, cfg.C), np.float32)
    for core in range(cfg.N_CORES):
        b, half = core // 2, core % 2
        d0 = half * cfg.D_LOC
        out[b, d0:d0 + cfg.D_LOC] = np.asarray(
            res.results[core]["out"], dtype=np.float32)
    return out


# revision 10
# speedup vs baseline: 1.3590x; 1.0051x over previous
"""CBAM3D Trainium2 kernel (8 NeuronCores, SPMD).

Reference computation (per batch sample b):
  avg_pool[c] = mean_{d,h,w} x ; max_pool[c] = max_{d,h,w} x
  ca = sigmoid(relu(avg@w1+b1)@w2+b2) + sigmoid(relu(max@w1+b1)@w2+b2)
  refined = x * ca[c]
  P = [mean_c refined, max_c refined]            # [D,H,W,2]
  sa = sigmoid(conv3d_same(P, conv_w))           # 7x7x7x2 -> 1
  out = refined * sa

Sharding: core i handles sample b=i//2, D-half half=i%2 (32 planes, NO host
halo padding). Cross-core traffic: pair-wise AllGathers of channel sum/max
stats (512B x2, split A/B so the first hides under the pass1 DMA window)
and of the 3-slot pooled-map halo (~108KB).

MEASURED ENGINE FACTS (HW, not the CoreSim model):
- DVE TensorTensor bf16 packed = 2x mode ~238 G out-elem/s; the ceiling.
  scalar_tensor_tensor (TensorScalarPtr) runs at 1x on real HW despite
  the cost model advertising 4x_2p — do NOT use it for bulk work.
- GpSimd bulk copy/tensor_scalar: 9-30 G elem/s (Q7 software) and the
  Pool engine REJECTS TensorTensor at the ISA level — no offload there.
- vector.tensor_reduce: ~120 G reads/s — slower than a TT halving tree;
  only worth it for the final 4->1 max fold (one small instruction).
- Per-op overhead ~90-130ns + tile semaphores; in-place acc ~equal to
  ping-pong (the "in-place penalty" was really just this overhead).
- ACT (scalar) engine: ~141 G elem/s copies, per-partition scale only.

Per-core pipeline (DVE is the wall; engine balance is the whole game):
  pass1: stream x f32 (plane-pair tiles), cast to a bf16 SBUF cache on
         ACT (wh0) / DVE (wh1), channel sum via PE matmul vs ones,
         channel max as running TT-max. STATS SPLIT: pairs 0..14 fold ->
         AllGather A launches ~t=100us (hidden under the DMA window);
         pair 15 has its own acc/psum -> tiny fold -> AllGather B right
         after the last cast. The serial barrier is then only B's
         latency + MLP instead of fold+AG+MLP after ALL of pass1.
  MLP: transpose-free tiny MLP on device -> ca (combines A+B stats)
  phase2a per pair: refined = cache*ca in-place; SUM tree C 64->4 and
         MAX tree C 64->4 (TT halving, DVE) + reduce-max 4->1; perm
         matmuls (PE) fold the 4 sum groups via PSUM accumulation and
         route (plane,h)->(ci,h') -> one ACT psum->pooled copy per slot.
         Edge pairs first; then the pooled halo exchange.
  conv:  49 taps x 5 blocks of accumulating matmuls with host-prebuilt
         band matrices (kh,ci folded into K=128) -> sigmoid -> sa stored
         C-pair-duplicated (innermost len-2 real stride keeps the apply
         TT in 2x mode; a stride-0 inner broadcast forces 1x)
  apply: cache *= sa in-place (TT 2x), one DMA per plane-pair to HBM bf16
"""

from dataclasses import dataclass

import numpy as np
import ml_dtypes

import concourse.bass as bass
import concourse.tile as tile
import concourse.mybir as mybir
from concourse import bacc, bass_isa

F32 = mybir.dt.float32
BF16 = mybir.dt.bfloat16
AX = mybir.AxisListType
OP = mybir.AluOpType
ACT = mybir.ActivationFunctionType


@dataclass(frozen=True)
class Cfg:
    H: int = 64
    W: int = 64
    C: int = 64
    D_LOC: int = 32          # own planes per core
    HID: int = 4             # C // reduction_ratio
    KS: int = 7
    N_CORES: int = 8
    use_collectives: bool = True
    stop_after: str = "full"   # pass1 | mlp | full

    @property
    def HALO(self):
        return self.KS // 2

    @property
    def S(self):
        return self.D_LOC + 2 * self.HALO   # slots in the pooled map

    @property
    def P(self):
        return 2 * self.H                    # partition dim of pair tiles

    @property
    def WP(self):
        return self.W + 2 * self.HALO        # padded pooled-map width

    @property
    def D_TOT(self):
        return 2 * self.D_LOC                # full-sample depth (2 shards)


FULL = Cfg()


def _bc(ap, shape, axis):
    """broadcast ap (by unsqueezing `axis`) to `shape`"""
    return ap.unsqueeze(axis).broadcast_to(shape)


def build_cbam(nc, cfg: Cfg):
    H, W, C = cfg.H, cfg.W, cfg.C
    P, S, WP, HALO = cfg.P, cfg.S, cfg.WP, cfg.HALO
    D_LOC, HID, KS = cfg.D_LOC, cfg.HID, cfg.KS
    PAIRS = D_LOC // 2
    BLK = 8
    W2 = W // 2
    NT = KS * KS
    MEAN_SCALE = 1.0 / float(2 * D_LOC * H * W)

    def tt(out, in0, in1, op):
        nc.vector.tensor_tensor(out=out, in0=in0, in1=in1, op=op)

    xs = nc.dram_tensor("xs", [D_LOC, H, W, C], F32, kind="ExternalInput").ap()
    w1 = nc.dram_tensor("w1", [C, HID], F32, kind="ExternalInput").ap()
    b1t = nc.dram_tensor("b1t", [HID, 1], F32, kind="ExternalInput").ap()
    w2 = nc.dram_tensor("w2", [HID, C], F32, kind="ExternalInput").ap()
    b2 = nc.dram_tensor("b2", [1, C], F32, kind="ExternalInput").ap()
    sband = nc.dram_tensor("sband", [P, NT, H], BF16, kind="ExternalInput").ap()
    routem = nc.dram_tensor("routem", [P, 4, P], BF16, kind="ExternalInput").ap()
    out_t = nc.dram_tensor("out", [D_LOC, H, W, C], BF16, kind="ExternalOutput").ap()

    groups = [[i, i + 1] for i in range(0, cfg.N_CORES, 2)]

    with tile.TileContext(nc) as tc:
        with (
            tc.tile_pool(name="consts", bufs=1) as consts,
            tc.tile_pool(name="cache", bufs=1) as cachep,
            tc.tile_pool(name="stage", bufs=5) as stagep,
            tc.tile_pool(name="tree", bufs=1) as treep,
            tc.tile_pool(name="work", bufs=2) as workp,
            tc.tile_pool(name="dram", bufs=1, space="DRAM") as dram,
            tc.tile_pool(name="ps_stats", bufs=2, space="PSUM") as ps_stats,
            tc.tile_pool(name="ps_perm", bufs=2, space="PSUM") as ps_perm,
            tc.tile_pool(name="ps_psp", bufs=1, space="PSUM") as ps_psp,
            tc.tile_pool(name="ps_cv", bufs=2, space="PSUM") as ps_cv,
            tc.tile_pool(name="ps_sm", bufs=1, space="PSUM") as ps_sm,
        ):
            # ---------------- constants ----------------
            ones = consts.tile([P, 1], BF16, tag="ones")
            nc.vector.memset(ones, 1.0)

            # perm matrices (host-built, see make_routem):
            #   0: qa_e p<64->col p      1: qb_e p<64->col p+64
            #   2: qa_o p>=64->col p-64  3: qb_o p>=64->col p
            rt_sb = consts.tile([P, 4, P], BF16, tag="routem")
            nc.gpsimd.dma_start(
                out=rt_sb[:].rearrange("p i q -> p (i q)"),
                in_=routem.rearrange("p i q -> p (i q)"))

            sband_sb = consts.tile([P, NT, H], BF16, tag="sband")
            nc.gpsimd.dma_start(
                out=sband_sb[:].rearrange("p t h -> p (t h)"),
                in_=sband.rearrange("p t h -> p (t h)"))
            w1_sb = consts.tile([C, HID], F32, tag="w1")
            nc.gpsimd.dma_start(out=w1_sb, in_=w1)
            w2_sb = consts.tile([HID, C], F32, tag="w2")
            nc.gpsimd.dma_start(out=w2_sb, in_=w2)
            b1t_sb = consts.tile([HID, 1], F32, tag="b1t")
            nc.gpsimd.dma_start(out=b1t_sb, in_=b1t)

            def dma_bcast(dst, src_ap, parts):
                a = bass.AP(tensor=src_ap.tensor, offset=src_ap.offset,
                            ap=[[0, parts]] + [list(p) for p in src_ap.ap[1:]])
                nc.gpsimd.dma_start(out=dst, in_=a)

            b2b = consts.tile([2, C], F32, tag="b2")
            dma_bcast(b2b, b2, 2)

            # pre-warm the ACT table set (Relu/Sigmoid) so the first real
            # activation in the latency-critical MLP doesn't pay the load
            warm = consts.tile([1, 1], F32, tag="warm")
            nc.scalar.activation(out=warm, in_=b2b[0:1, 0:1], func=ACT.Relu)
            nc.scalar.activation(out=warm, in_=warm, func=ACT.Sigmoid)
            ones12 = consts.tile([1, 2], F32, tag="ones12")
            nc.vector.memset(ones12, 1.0)

            if cfg.use_collectives:
                wu_s = dram.tile([1, 1], F32, tag="wu_s")
                wu_r = dram.tile([2, 1], F32, tag="wu_r")
                nc.gpsimd.dma_start(out=wu_s, in_=b2b[0:1, 0:1])
                nc.gpsimd.collective_compute(
                    "AllGather", OP.bypass, replica_groups=groups,
                    ins=[wu_s.opt()], outs=[wu_r.opt()])

            # persistent state. pair j covers planes (2j, 2j+1) -> pooled
            # slots (HALO+2j, HALO+2j+1). Halo slots 0:3 / 35:38 come from
            # the neighbor core (or stay zero at sample boundaries).
            cache = [cachep.tile([P, W, C], BF16, tag=f"cache{j}",
                                 name=f"cache{j}") for j in range(PAIRS)]
            W4 = W // 4
            # stats-split accumulators: A covers pairs 0..14, B pair 15
            acc_a = cachep.tile([P, W4, C], BF16, tag="acc_a")
            nc.vector.memset(acc_a, -3.0e38)
            acc_b = cachep.tile([P, W4, C], BF16, tag="acc_b")
            nc.vector.memset(acc_b, -3.0e38)
            pooled = cachep.tile([P, S, WP], BF16, tag="pooled")
            nc.gpsimd.memset(pooled, 0.0)
            # conv blocks (start plane, size): the final 8 planes are two
            # 4-plane blocks so the last tree->conv->apply tail is shorter
            conv_blocks = [(0, 8), (8, 8), (16, 8), (24, 4), (28, 4)]
            sa_sb = [cachep.tile([H, sz, W], BF16, tag=f"sa{b}", name=f"sa{b}")
                     for b, (_, sz) in enumerate(conv_blocks)]
            # sa duplicated along a trailing len-2 axis: the apply
            # tensor_tensor then reads packed bf16 pairs (2x DVE mode).
            sa_dup = [cachep.tile([P, sz // 2, W, 2], BF16, tag=f"sad{b}",
                                  name=f"sad{b}")
                      for b, (_, sz) in enumerate(conv_blocks)]

            # ---------------- pass 1: stream + cast + stats ----------------
            psum_a = ps_stats.tile([1, 8, C], F32, tag="stats", name="statsA")
            psum_b = ps_stats.tile([1, 8, C], F32, tag="stats", name="statsB")
            n_wg = W // 8

            def fold_stats(psum, acc, sname, mname):
                """psum [1,8,C] + acc [P,W4,C] -> sumc [1,C], maxr [P,C]"""
                s8 = workp.tile([1, 8, C], F32, tag=f"s8{sname}", bufs=1)
                nc.scalar.copy(out=s8, in_=psum)
                nc.vector.tensor_add(out=s8[:, 0:4, :], in0=s8[:, 0:4, :],
                                     in1=s8[:, 4:8, :])
                nc.vector.tensor_add(out=s8[:, 0:2, :], in0=s8[:, 0:2, :],
                                     in1=s8[:, 2:4, :])
                sumc = workp.tile([1, C], F32, tag=f"sumc{sname}", bufs=1)
                nc.vector.tensor_add(out=sumc, in0=s8[:, 0, :],
                                     in1=s8[:, 1, :])
                nc.scalar.mul(out=sumc, in_=sumc, mul=MEAN_SCALE)
                wfold = W4
                while wfold > 1:
                    wfold //= 2
                    tt(acc[:, 0:wfold, :].rearrange("p w c -> p (w c)"),
                       acc[:, 0:wfold, :].rearrange("p w c -> p (w c)"),
                       acc[:, wfold:2 * wfold, :]
                       .rearrange("p w c -> p (w c)"), OP.max)
                maxr = workp.tile([P, C], F32, tag=f"maxr{mname}", bufs=1)
                nc.gpsimd.partition_all_reduce(
                    out_ap=maxr, in_ap=acc[:, 0, :], channels=P,
                    reduce_op=bass_isa.ReduceOp.max)
                return sumc, maxr

            def emit_pass1_pair(j, psum, acc, mm_start, mm_stop):
                for wh in range(2):
                    st = stagep.tile([P, W2, C], F32, tag="stage")
                    nc.sync.dma_start(
                        out=st.rearrange("p w c -> p (w c)"),
                        in_=xs[2 * j:2 * j + 2, :, wh * W2:(wh + 1) * W2, :]
                        .rearrange("d h w c -> (d h) (w c)"))
                    if wh == 0:
                        nc.scalar.copy(out=cache[j][:, 0:W2, :], in_=st)
                    else:
                        nc.vector.tensor_copy(out=cache[j][:, W2:, :], in_=st)
                    for qq in range(2):
                        q0 = wh * W2 + qq * W4
                        tt(acc[:].rearrange("p w c -> p (w c)"),
                           acc[:].rearrange("p w c -> p (w c)"),
                           cache[j][:, q0:q0 + W4, :]
                           .rearrange("p w c -> p (w c)"), OP.max)
                for g in range(n_wg):
                    nc.tensor.matmul(
                        out=psum, lhsT=ones[:, :],
                        rhs=cache[j][:, g * 8:(g + 1) * 8, :],
                        start=(mm_start and g == 0),
                        stop=(mm_stop and g == n_wg - 1))

            for j in range(PAIRS - 1):
                emit_pass1_pair(j, psum_a, acc_a,
                                mm_start=(j == 0), mm_stop=(j == PAIRS - 2))

            # ---- stats A: fold + AllGather, hidden under the DMA window
            sumc_a, maxr_a = fold_stats(psum_a, acc_a, "a", "a")
            snd_a = dram.tile([2, C], F32, tag="snd_a")
            rcv_a = dram.tile([2, 2, C], F32, tag="rcv_a")
            nc.sync.dma_start(out=snd_a[0:1, :], in_=sumc_a)
            nc.sync.dma_start(out=snd_a[1:2, :], in_=maxr_a[0:1, :])
            if cfg.use_collectives:
                nc.gpsimd.collective_compute(
                    "AllGather", OP.bypass, replica_groups=groups,
                    ins=[snd_a.opt()], outs=[rcv_a.opt()])
            else:
                nc.gpsimd.dma_start(out=rcv_a[0], in_=snd_a)
                nc.gpsimd.dma_start(out=rcv_a[1], in_=snd_a)

            # ---- last pair -> stats B (short critical chain)
            emit_pass1_pair(PAIRS - 1, psum_b, acc_b,
                            mm_start=True, mm_stop=True)
            sumc_b, maxr_b = fold_stats(psum_b, acc_b, "b", "b")
            snd_b = dram.tile([2, C], F32, tag="snd_b")
            rcv_b = dram.tile([2, 2, C], F32, tag="rcv_b")
            nc.sync.dma_start(out=snd_b[0:1, :], in_=sumc_b)
            nc.sync.dma_start(out=snd_b[1:2, :], in_=maxr_b[0:1, :])
            if cfg.use_collectives:
                nc.gpsimd.collective_compute(
                    "AllGather", OP.bypass, replica_groups=groups,
                    ins=[snd_b.opt()], outs=[rcv_b.opt()])
            else:
                nc.gpsimd.dma_start(out=rcv_b[0], in_=snd_b)
                nc.gpsimd.dma_start(out=rcv_b[1], in_=snd_b)

            # ---------------- MLP -> ca (transpose-free) ----------------
            if cfg.stop_after == "pass1":
                return nc
            # land stats transposed: quadT[c, k, g*2+r] = rcv_g[r, k, c]
            quadT = workp.tile([C, 2, 4], F32, tag="quadT", bufs=1)
            for g, rcv in enumerate((rcv_a, rcv_b)):
                for r in range(2):
                    nc.sync.dma_start(out=quadT[:, :, 2 * g + r],
                                      in_=rcv[r].rearrange("k c -> c k"))
            pooled2 = workp.tile([C, 2], F32, tag="pooled2", bufs=1)
            # sum row: A0+A1+B0+B1 ; max row: max of the four
            nc.vector.tensor_add(out=pooled2[:, 0:1], in0=quadT[:, 0, 0:1],
                                 in1=quadT[:, 0, 1:2])
            nc.vector.tensor_add(out=pooled2[:, 0:1], in0=pooled2[:, 0:1],
                                 in1=quadT[:, 0, 2:3])
            nc.vector.tensor_add(out=pooled2[:, 0:1], in0=pooled2[:, 0:1],
                                 in1=quadT[:, 0, 3:4])
            nc.vector.tensor_tensor(out=pooled2[:, 1:2], in0=quadT[:, 1, 0:1],
                                    in1=quadT[:, 1, 1:2], op=OP.max)
            nc.vector.tensor_tensor(out=pooled2[:, 1:2], in0=pooled2[:, 1:2],
                                    in1=quadT[:, 1, 2:3], op=OP.max)
            nc.vector.tensor_tensor(out=pooled2[:, 1:2], in0=pooled2[:, 1:2],
                                    in1=quadT[:, 1, 3:4], op=OP.max)

            psum_h = ps_sm.tile([HID, 2], F32, tag="small")
            nc.tensor.matmul(out=psum_h, lhsT=w1_sb, rhs=pooled2,
                             start=True, stop=True)
            h2 = workp.tile([HID, 2], F32, tag="h2", bufs=1)
            nc.scalar.activation(out=h2, in_=psum_h, func=ACT.Relu,
                                 bias=b1t_sb)
            # psum_ca = h2.T @ w2 + 1x2.T @ b2 (bias folded in as a matmul)
            psum_ca = ps_sm.tile([2, C], F32, tag="small")
            nc.tensor.matmul(out=psum_ca, lhsT=h2, rhs=w2_sb,
                             start=True, stop=False)
            nc.tensor.matmul(out=psum_ca, lhsT=ones12, rhs=b2b[0:1, :],
                             start=False, stop=True)
            ca2 = workp.tile([2, C], BF16, tag="ca2", bufs=1)
            nc.scalar.activation(out=ca2, in_=psum_ca, func=ACT.Sigmoid)
            car = workp.tile([2, C], BF16, tag="car", bufs=1)
            nc.gpsimd.partition_all_reduce(
                out_ap=car, in_ap=ca2, channels=2,
                reduce_op=bass_isa.ReduceOp.add)
            ca_bf = consts.tile([P, C], BF16, tag="ca_bf")
            nc.gpsimd.partition_broadcast(out_ap=ca_bf, in_ap=car[0:1, :])

            # ---------------- phase 2: pooled + conv + apply ----------------
            if cfg.stop_after == "mlp":
                return nc

            def emit_pair_phase2a(j):
                """refine in-place; SUM tree to [P,W,4] + MAX tree to [P,W]
                (DVE TT); perm matmuls (PE) fold the final 4-way sum via
                PSUM accumulation; psum->pooled copies (ACT)."""
                s_e, s_o = HALO + 2 * j, HALO + 2 * j + 1
                tt(cache[j], cache[j], _bc(ca_bf[:, :], [P, W, C], 1),
                   OP.mult)
                # SUM tree: halve C 64 -> 4 (stays 2x throughout)
                t1s = treep.tile([P, W, C // 2], BF16, tag="t1add",
                                 name=f"t1add_{j}")
                with nc.allow_low_precision(reason="bf16 pooled stats"):
                    tt(t1s, cache[j][:, :, 0:C // 2],
                       cache[j][:, :, C // 2:], OP.add)
                    cf = C // 2
                    while cf > 4:
                        cf //= 2
                        tt(t1s[:, :, 0:cf], t1s[:, :, 0:cf],
                           t1s[:, :, cf:2 * cf], OP.add)
                # MAX tree: halve C 64 -> 4, then reduce 4 -> 1
                t1m = treep.tile([P, W, C // 2], BF16, tag="t1max",
                                 name=f"t1max_{j}")
                rpm = workp.tile([P, W], BF16, tag="rpmax",
                                 name=f"rpmax_{j}")
                tt(t1m, cache[j][:, :, 0:C // 2],
                   cache[j][:, :, C // 2:], OP.max)
                cf = C // 2
                while cf > 4:
                    cf //= 2
                    tt(t1m[:, :, 0:cf], t1m[:, :, 0:cf],
                       t1m[:, :, cf:2 * cf], OP.max)
                nc.vector.tensor_reduce(
                    out=rpm, in_=t1m[:, :, 0:4], axis=AX.X, op=OP.max)
                # perm matmuls; the 4 leftover sum groups accumulate in PSUM
                for qa, qb, slot, nm in ((0, 1, s_e, "pe"),
                                         (2, 3, s_o, "po")):
                    pp = ps_perm.tile([P, W], F32, tag="perm",
                                      name=f"{nm}{j}")
                    for k in range(4):
                        nc.tensor.matmul(out=pp, lhsT=rt_sb[:, qa, :],
                                         rhs=t1s[:, :, k],
                                         start=(k == 0), stop=False)
                    nc.tensor.matmul(out=pp, lhsT=rt_sb[:, qb, :], rhs=rpm,
                                     start=False, stop=True)
                    nc.scalar.copy(out=pooled[:, slot, HALO:HALO + W], in_=pp)

            # edge pairs first: they feed the pooled-halo exchange
            pair_order = [0, 1, PAIRS - 2, PAIRS - 1] + list(range(2, PAIRS - 2))
            emitted = 0
            while emitted < 4:
                emit_pair_phase2a(pair_order[emitted])
                emitted += 1

            # ---- pooled-map halo exchange (pair-wise) ----
            snd_h = dram.tile([P, 6 * WP], BF16, tag="snd_h")
            rcv_h = dram.tile([2, P, 6 * WP], BF16, tag="rcv_h")
            nc.sync.dma_start(
                out=snd_h[:, 0:3 * WP],
                in_=pooled[:, HALO:2 * HALO, :].rearrange("p s w -> p (s w)"))
            nc.sync.dma_start(
                out=snd_h[:, 3 * WP:],
                in_=pooled[:, S - 2 * HALO:S - HALO, :]
                .rearrange("p s w -> p (s w)"))
            if cfg.use_collectives:
                nc.gpsimd.collective_compute(
                    "AllGather", OP.bypass, replica_groups=groups,
                    ins=[snd_h.opt()], outs=[rcv_h.opt()])
            else:
                nc.gpsimd.dma_start(out=rcv_h[0], in_=snd_h)
                nc.gpsimd.dma_start(out=rcv_h[1], in_=snd_h)
            par = nc.sync.partition_id() & 1
            # half 0: my top halo slots <- neighbor's first 3 own planes
            nc.sync.dma_start(
                out=pooled[:, S - HALO:S, :].rearrange("p s w -> p (s w)"),
                in_=rcv_h[1, :, 0:3 * WP], cond=1 - par)
            # half 1: my low halo slots <- neighbor's last 3 own planes
            nc.sync.dma_start(
                out=pooled[:, 0:HALO, :].rearrange("p s w -> p (s w)"),
                in_=rcv_h[0, :, 3 * WP:], cond=par)

            def emit_conv_blk(blk, start, sz):
                pcv = ps_cv.tile([H, sz, W], F32, tag="cv", name=f"cv{blk}")
                k = 0
                for kd in range(KS):
                    for kw in range(KS):
                        nc.tensor.matmul(
                            out=pcv,
                            lhsT=sband_sb[:, kd * KS + kw, :],
                            rhs=pooled[:, start + kd: start + kd + sz,
                                       kw:kw + W],
                            start=(k == 0), stop=(k == NT - 1),
                            skip_group_check=True)
                        k += 1
                nc.scalar.activation(out=sa_sb[blk], in_=pcv, func=ACT.Sigmoid)
                sa_ev = sa_sb[blk].rearrange("h (a b) w -> h a b w", b=2)
                psp = ps_psp.tile([P, sz // 2, W], F32, tag="psp",
                                  name=f"psp{blk}")
                nc.tensor.matmul(out=psp, lhsT=rt_sb[0:H, 0, :],
                                 rhs=sa_ev[:, :, 0, :], start=True, stop=False)
                nc.tensor.matmul(out=psp, lhsT=rt_sb[0:H, 1, :],
                                 rhs=sa_ev[:, :, 1, :], start=False, stop=True)
                # duplicate along a trailing len-2 axis while copying out
                nc.scalar.copy(
                    out=sa_dup[blk],
                    in_=_bc(psp, [P, sz // 2, W, 2], 3))

            def emit_applies(blk, start, sz):
                for j in range(start // 2, start // 2 + sz // 2):
                    dp = j - start // 2
                    cv = cache[j].rearrange("p w (a b) -> p w a b", b=2)
                    nc.vector.tensor_tensor(
                        out=cv, in0=cv,
                        in1=_bc(sa_dup[blk][:, dp], [P, W, C // 2, 2], 2),
                        op=OP.mult)
                    nc.sync.dma_start(
                        out=out_t[2 * j:2 * j + 2]
                        .rearrange("d h w c -> (d h) (w c)"),
                        in_=cache[j].rearrange("p w c -> p (w c)"))

            # applies are deferred one conv block: engines run in program
            # order, so an apply emitted right after its conv would stall
            # DVE on the PE pipeline while tree work is still available
            need_emit = [8, 12, 16, 16, 16]  # pairs done before conv blk
            prev = None
            for blk, (start, sz) in enumerate(conv_blocks):
                while emitted < need_emit[blk]:
                    emit_pair_phase2a(pair_order[emitted])
                    emitted += 1
                emit_conv_blk(blk, start, sz)
                if prev is not None:
                    emit_applies(*prev)
                prev = (blk, start, sz)
            emit_applies(*prev)
    return nc


def make_sband(conv_w, cfg: Cfg):
    """Host-side band-matrix construction: [P, KS*KS, H] bf16.

    sband[ci*H+h', kd*KS+kw, h] = conv_w[kd, h'-h+halo, kw, ci] (avg rows
    pre-scaled by 1/C because the pooled map stores channel sums)."""
    H, C, KS, HALO = cfg.H, cfg.C, cfg.KS, cfg.HALO
    cw = np.asarray(conv_w, np.float32)[..., 0]        # [KS,KS,KS,2]
    sb = np.zeros((cfg.P, KS * KS, H), np.float32)
    h = np.arange(H)
    for kd in range(KS):
        for kw in range(KS):
            for ci in range(2):
                scale = (1.0 / C) if ci == 0 else 1.0
                for kh in range(KS):
                    hp = h + kh - HALO                  # h' = h + kh - halo
                    m = (hp >= 0) & (hp < H)
                    sb[ci * H + hp[m], kd * KS + kw, h[m]] = cw[kd, kh, kw, ci] * scale
    return sb.astype(ml_dtypes.bfloat16)


def make_routem(cfg: Cfg):
    """Perm matrices [P, 4, P] bf16 (lhsT convention: out[q] sums
    lhsT[p, q] * rhs[p]).

    i=0 (qa_e): p<64  -> col p     (even plane -> avg rows / sa even)
    i=1 (qb_e): p<64  -> col p+64  (even plane -> max rows / sa odd)
    i=2 (qa_o): p>=64 -> col p-64  (odd plane -> avg rows)
    i=3 (qb_o): p>=64 -> col p     (odd plane -> max rows)
    """
    P, H = cfg.P, cfg.H
    rm = np.zeros((P, 4, P), np.float32)
    h = np.arange(H)
    rm[h, 0, h] = 1.0
    rm[h, 1, h + H] = 1.0
    rm[H + h, 2, h] = 1.0
    rm[H + h, 3, H + h] = 1.0
    return rm.astype(ml_dtypes.bfloat16)


def make_core_inputs(x, w1, b1, w2, b2, sband_np, routem_np, cfg: Cfg):
    """Shard the full inputs into per-core in_maps (no halo padding)."""
    C, D_LOC = cfg.C, cfg.D_LOC
    x = np.ascontiguousarray(np.asarray(x, np.float32))
    in_maps = []
    for core in range(cfg.N_CORES):
        b, half = core // 2, core % 2
        d0 = half * D_LOC
        in_maps.append({
            "xs": x[b, d0:d0 + D_LOC],
            "w1": np.asarray(w1, np.float32).reshape(C, cfg.HID),
            "b1t": np.asarray(b1, np.float32).reshape(cfg.HID, 1),
            "w2": np.asarray(w2, np.float32).reshape(cfg.HID, C),
            "b2": np.asarray(b2, np.float32).reshape(1, C),
            "sband": sband_np,
            "routem": routem_np,
        })
    return in_maps


_COMPILED = {}


def get_compiled(cfg: Cfg = FULL):
    if cfg not in _COMPILED:
        nc = bacc.Bacc("TRN2", target_bir_lowering=False, debug=False,
                       num_devices=cfg.N_CORES)
        build_cbam(nc, cfg)
        nc.compile()
        _COMPILED[cfg] = nc
    return _COMPILED[cfg]


def kernel(x, w1, b1, w2, b2, conv_w):
    from concourse.bass_utils import run_bass_kernel_spmd

    cfg = FULL
    nc = get_compiled(cfg)
    sband_np = make_sband(conv_w, cfg)
    routem_np = make_routem(cfg)
    in_maps = make_core_inputs(x, w1, b1, w2, b2, sband_np, routem_np, cfg)
    res = run_bass_kernel_spmd(nc, in_maps, list(range(cfg.N_CORES)))
    B, D = 4, 64
    out = np.empty((B, D, cfg.H, cfg.W, cfg.C), np.float32)
    for core in range(cfg.N_CORES):
        b, half = core // 2, core % 2
        d0 = half * cfg.D_LOC
        out[b, d0:d0 + cfg.D_LOC] = np.asarray(
            res.results[core]["out"], dtype=np.float32)
    return out
